# revision 11
# baseline (speedup 1.0000x reference)
"""CutMix kernel for Trainium2, data-parallel over batch across 8 NeuronCores.

Semantics (matches the jax reference):
  crop_len = int32((1 - lam) * L)
  mask[b, x] = (x >= start[b]) & (x < start[b] + crop_len[b]) & (dec[b] == 1)
  wave_mix = where(mask, wave[perm], wave)
  label_out = dec * (lam * onehot + (1 - lam) * onehot[perm]) + (1 - dec) * onehot

Strategy: host plans row assignment from the tiny control tensors
(lam/perm/dec/start); the 82 MB wave traffic all happens on device.
Each core owns 16 rows: rows that need mixing ("select rows", K per core,
host-balanced) stream through SBUF as [128, 1250] tiles and get a
predicated copy from the permuted-source row; untouched rows go through
one big DRAM->DRAM DMA. The label mix runs on every core (tiny,
uniform program) via a one-hot permutation matmul on the PE; core 0's
result is used.
"""

import os
import sys
from contextlib import ExitStack

import numpy as np

try:
    from concourse import bacc, bass, tile
    from concourse.bass_utils import run_bass_kernel_spmd
except ImportError:  # grading container path
    sys.path.insert(0, "/opt/trn_rl_repo")
    from concourse import bacc, bass, tile
    from concourse.bass_utils import run_bass_kernel_spmd

B, L, C = 128, 160000, 128
NCORES = 8
ROWS_PER_CORE = B // NCORES  # 16
PPART = 128                  # SBUF partitions used per row tile
FREE = L // PPART            # 1250 f32 per partition
BIG = np.float32(2 ** 25)    # sentinel bound: idx < 2^25 always, mask stays 0

_PROGRAM_CACHE: dict[int, "bass.Bass"] = {}
LAST_RESULTS = None  # stashed BassKernelResults for test harness inspection


def _build_program(K: int, repeat: int = 1) -> "bass.Bass":
    """One SPMD program, parameterized only by K = select-rows per core.

    repeat > 1 replays the whole body (timing amplification only).
    """
    nc = bacc.Bacc("TRN2", target_bir_lowering=False)
    dt = bass.mybir.dt
    op = bass.mybir.AluOpType
    NI = ROWS_PER_CORE - K  # identity rows per core

    i_idx = nc.dram_tensor("idx", [PPART, FREE], dt.float32, kind="ExternalInput")
    i_lbl = nc.dram_tensor("lbl", [C, 2 * C + 2], dt.float32, kind="ExternalInput")
    o_lab = nc.dram_tensor("o_lab", [C, C], dt.float32, kind="ExternalOutput")
    if K > 0:
        i_sown = nc.dram_tensor(
            "sown", [K, PPART, FREE], dt.float32, kind="ExternalInput"
        )
        i_sprm = nc.dram_tensor(
            "sprm", [K, PPART, FREE], dt.float32, kind="ExternalInput"
        )
        i_bnd = nc.dram_tensor("bnd", [PPART, 2 * K], dt.float32, kind="ExternalInput")
        o_sel = nc.dram_tensor(
            "o_sel", [K, PPART, FREE], dt.float32, kind="ExternalOutput"
        )
    if NI > 0:
        i_idn = nc.dram_tensor("idn", [NI, L], dt.float32, kind="ExternalInput")
        o_idn = nc.dram_tensor("o_idn", [NI, L], dt.float32, kind="ExternalOutput")

    with tile.TileContext(nc) as tc, ExitStack() as ctx:
        const = ctx.enter_context(tc.tile_pool(name="const", bufs=1))
        work = ctx.enter_context(tc.tile_pool(name="work", bufs=4))
        psum = ctx.enter_context(tc.tile_pool(name="psum", bufs=1, space="PSUM"))

        for _rep in range(repeat):
            # Untouched rows: straight DRAM->DRAM copies, one per row so
            # the transfers spread across DMA queues.
            if NI > 0:
                for r in range(NI):
                    nc.sync.dma_start(o_idn[r : r + 1, :], i_idn[r : r + 1, :])

            # Label mix: oh_perm = pmatT.T @ oh (exact: one-hot rows), then
            # lab = c0 * oh + c1 * oh_perm with per-partition scalars.
            t_lbl = const.tile([C, 2 * C + 2], dt.float32, tag="lbl")
            nc.gpsimd.dma_start(t_lbl[:], i_lbl[:])
            t_oh = t_lbl[:, 0:C]
            t_pm = t_lbl[:, C : 2 * C]
            p_lab = psum.tile([C, C], dt.float32, tag="plab")
            nc.tensor.matmul(p_lab[:], t_pm, t_oh, start=True, stop=True)
            t_l0 = const.tile([C, C], dt.float32, tag="l0")
            nc.vector.tensor_scalar(
                t_l0[:], t_oh, t_lbl[:, 2 * C : 2 * C + 1], None, op.mult
            )
            t_l1 = const.tile([C, C], dt.float32, tag="l1")
            nc.vector.tensor_scalar(
                t_l1[:], p_lab[:], t_lbl[:, 2 * C + 1 : 2 * C + 2], None, op.mult
            )
            nc.vector.tensor_add(t_l0[:], t_l0[:], t_l1[:])
            nc.sync.dma_start(o_lab[:], t_l0[:])

            # Select rows: mask = (idx >= s) - (idx >= e), then predicated
            # copy of the permuted-source tile over the own tile.
            if K > 0:
                t_idx = const.tile([PPART, FREE], dt.float32, tag="idx")
                nc.gpsimd.dma_start(t_idx[:], i_idx[:])
                t_bnd = const.tile([PPART, 2 * K], dt.float32, tag="bnd")
                nc.gpsimd.dma_start(t_bnd[:], i_bnd[:])
                for r in range(K):
                    t_own = work.tile([PPART, FREE], dt.float32, tag="own")
                    nc.sync.dma_start(t_own[:], i_sown[r, :, :])
                    t_prm = work.tile([PPART, FREE], dt.float32, tag="prm")
                    nc.sync.dma_start(t_prm[:], i_sprm[r, :, :])
                    m_e = work.tile([PPART, FREE], dt.float32, tag="m_e")
                    nc.vector.tensor_scalar(
                        m_e[:],
                        t_idx[:],
                        t_bnd[:, 2 * r + 1 : 2 * r + 2],
                        None,
                        op.is_ge,
                    )
                    m = work.tile([PPART, FREE], dt.int32, tag="m")
                    nc.vector.scalar_tensor_tensor(
                        m[:],
                        t_idx[:],
                        t_bnd[:, 2 * r : 2 * r + 1],
                        m_e[:],
                        op.is_ge,
                        op.subtract,
                    )
                    nc.vector.copy_predicated(t_own[:], m[:], t_prm[:])
                    nc.sync.dma_start(o_sel[r, :, :], t_own[:])
    nc.compile()
    return nc


def plan_and_maps(wave, onehot, lam, perm, dec, start):
    wave = np.ascontiguousarray(wave, dtype=np.float32)
    onehot = np.ascontiguousarray(onehot, dtype=np.float32)
    lam = np.asarray(lam, dtype=np.float32)
    perm = np.asarray(perm, dtype=np.int32)
    dec = np.asarray(dec, dtype=np.float32)
    start = np.asarray(start, dtype=np.int32)

    # --- host planning from the small control tensors ---
    crop = ((np.float32(1.0) - lam) * np.float32(L)).astype(np.int32)
    end = start + crop
    is_mix = (dec == 1.0) & (crop > 0)
    mix_rows = np.nonzero(is_mix)[0]
    idn_rows = np.nonzero(~is_mix)[0]
    T = len(mix_rows)
    K = -(-T // NCORES)  # ceil
    NI = ROWS_PER_CORE - K

    # Deal mix rows round-robin, then pad each core's select list with
    # identity rows (sentinel bounds keep them bit-exact identity copies).
    sel: list[list[int]] = [list(mix_rows[c::NCORES]) for c in range(NCORES)]
    pool = list(idn_rows)
    for c in range(NCORES):
        while len(sel[c]) < K:
            sel[c].append(pool.pop())
    idn = [[pool.pop() for _ in range(NI)] for _ in range(NCORES)]
    assert not pool

    idx_np = np.arange(L, dtype=np.float32).reshape(PPART, FREE)
    pmatT = np.zeros((C, C), dtype=np.float32)
    pmatT[perm, np.arange(C)] = 1.0
    c0 = np.float32(1.0) - dec + dec * lam
    c1 = dec * (np.float32(1.0) - lam)
    cv = np.stack([c0, c1], axis=1).astype(np.float32)

    in_maps = []
    for c in range(NCORES):
        m = {
            "idx": idx_np,
            "lbl": np.concatenate([onehot, pmatT, cv], axis=1),
        }
        if K > 0:
            rows = sel[c]
            m["sown"] = wave[rows].reshape(K, PPART, FREE)
            m["sprm"] = wave[perm[rows]].reshape(K, PPART, FREE)
            bnd = np.empty((PPART, 2 * K), dtype=np.float32)
            for j, b in enumerate(rows):
                if is_mix[b]:
                    bnd[:, 2 * j] = start[b]
                    bnd[:, 2 * j + 1] = end[b]
                else:
                    bnd[:, 2 * j] = BIG
                    bnd[:, 2 * j + 1] = BIG
            m["bnd"] = bnd
        if NI > 0:
            m["idn"] = wave[idn[c]]
        in_maps.append(m)

    return K, NI, sel, idn, in_maps


def assemble(K, NI, sel, idn, per_core_outs):
    wave_mix = np.empty((B, L), dtype=np.float32)
    for c in range(NCORES):
        out = per_core_outs[c]
        if K > 0:
            o_sel = np.asarray(out["o_sel"]).reshape(K, L)
            for j, b in enumerate(sel[c]):
                wave_mix[b] = o_sel[j]
        if NI > 0:
            o_idn = np.asarray(out["o_idn"])
            for j, b in enumerate(idn[c]):
                wave_mix[b] = o_idn[j]
    label_out = np.asarray(per_core_outs[0]["o_lab"]).reshape(C, C)
    return wave_mix, label_out


def get_program(K):
    prog = _PROGRAM_CACHE.get(K)
    if prog is None:
        prog = _build_program(K)
        _PROGRAM_CACHE[K] = prog
    return prog


def kernel(wave, onehot, lam, perm, dec, start):
    global LAST_RESULTS
    K, NI, sel, idn, in_maps = plan_and_maps(wave, onehot, lam, perm, dec, start)
    prog = get_program(K)
    res = run_bass_kernel_spmd(prog, in_maps, core_ids=list(range(NCORES)))
    LAST_RESULTS = res
    return assemble(K, NI, sel, idn, res.results)


# revision 13
# speedup vs baseline: 7.1006x; 7.1006x over previous
"""CutMix kernel for Trainium2, data-parallel over batch across 8 NeuronCores.

Semantics (matches the jax reference):
  crop_len = int32((1 - lam) * L)
  mask[b, x] = (x >= start[b]) & (x < start[b] + crop_len[b]) & (dec[b] == 1)
  wave_mix = where(mask, wave[perm], wave)
  label_out = dec * (lam * onehot + (1 - lam) * onehot[perm]) + (1 - dec) * onehot

Strategy: host plans row assignment from the tiny control tensors
(lam/perm/dec/start); the 82 MB wave traffic all happens on device.
Each core owns 16 rows: rows that need mixing ("select rows", K per core,
host-balanced) stream through SBUF as [128, 1250] tiles and get a
predicated copy from the permuted-source row; untouched rows go through
one big DRAM->DRAM DMA. The label mix runs on every core (tiny,
uniform program) via a one-hot permutation matmul on the PE; core 0's
result is used.
"""

import os
import sys
from contextlib import ExitStack

import numpy as np

try:
    from concourse import bacc, bass, tile
    from concourse.bass_utils import run_bass_kernel_spmd
except ImportError:  # grading container path
    sys.path.insert(0, "/opt/trn_rl_repo")
    from concourse import bacc, bass, tile
    from concourse.bass_utils import run_bass_kernel_spmd

B, L, C = 128, 160000, 128
NCORES = 8
ROWS_PER_CORE = B // NCORES  # 16
PPART = 128                  # SBUF partitions used per row tile
FREE = L // PPART            # 1250 f32 per partition
BIG = np.float32(2 ** 25)    # sentinel bound: idx < 2^25 always, mask stays 0

_PROGRAM_CACHE: dict[int, "bass.Bass"] = {}
LAST_RESULTS = None  # stashed BassKernelResults for test harness inspection


def _build_program(K: int, repeat: int = 1) -> "bass.Bass":
    """One SPMD program, parameterized only by K = select-rows per core.

    repeat > 1 replays the whole body (timing amplification only).
    """
    nc = bacc.Bacc("TRN2", target_bir_lowering=False)
    dt = bass.mybir.dt
    op = bass.mybir.AluOpType
    NI = ROWS_PER_CORE - K  # identity rows per core

    i_idx = nc.dram_tensor("idx", [PPART, FREE], dt.float32, kind="ExternalInput")
    i_lbl = nc.dram_tensor("lbl", [C, 2 * C + 2], dt.float32, kind="ExternalInput")
    o_lab = nc.dram_tensor("o_lab", [C, C], dt.float32, kind="ExternalOutput")
    if K > 0:
        # Transposed "panel" layout: [128 partitions, K*1250] so one DMA
        # moves all K rows at full 128-partition DMA efficiency.
        i_sown = nc.dram_tensor(
            "sown", [PPART, K * FREE], dt.float32, kind="ExternalInput"
        )
        i_sprm = nc.dram_tensor(
            "sprm", [PPART, K * FREE], dt.float32, kind="ExternalInput"
        )
        i_bnd = nc.dram_tensor("bnd", [PPART, 2 * K], dt.float32, kind="ExternalInput")
        o_sel = nc.dram_tensor(
            "o_sel", [PPART, K * FREE], dt.float32, kind="ExternalOutput"
        )
    if NI > 0:
        # Natural row-major data reinterpreted as [NI*128, 1250] so the
        # DRAM->DRAM copy is descriptor-parallel.
        i_idn = nc.dram_tensor(
            "idn", [NI * PPART, FREE], dt.float32, kind="ExternalInput"
        )
        o_idn = nc.dram_tensor(
            "o_idn", [NI * PPART, FREE], dt.float32, kind="ExternalOutput"
        )

    with tile.TileContext(nc) as tc, ExitStack() as ctx:
        const = ctx.enter_context(tc.tile_pool(name="const", bufs=1))
        panel = ctx.enter_context(tc.tile_pool(name="panel", bufs=1))
        work = ctx.enter_context(tc.tile_pool(name="work", bufs=4))
        psum = ctx.enter_context(tc.tile_pool(name="psum", bufs=1, space="PSUM"))

        for _rep in range(repeat):
            # Untouched rows: one straight DRAM->DRAM copy.
            if NI > 0:
                nc.sync.dma_start(o_idn[:], i_idn[:])

            # Label mix: oh_perm = pmatT.T @ oh (exact: one-hot rows), then
            # lab = c0 * oh + c1 * oh_perm with per-partition scalars.
            t_lbl = const.tile([C, 2 * C + 2], dt.float32, tag="lbl")
            nc.gpsimd.dma_start(t_lbl[:], i_lbl[:])
            t_oh = t_lbl[:, 0:C]
            t_pm = t_lbl[:, C : 2 * C]
            p_lab = psum.tile([C, C], dt.float32, tag="plab")
            nc.tensor.matmul(p_lab[:], t_pm, t_oh, start=True, stop=True)
            t_l0 = const.tile([C, C], dt.float32, tag="l0")
            nc.vector.tensor_scalar(
                t_l0[:], t_oh, t_lbl[:, 2 * C : 2 * C + 1], None, op.mult
            )
            t_l1 = const.tile([C, C], dt.float32, tag="l1")
            nc.vector.tensor_scalar(
                t_l1[:], p_lab[:], t_lbl[:, 2 * C + 1 : 2 * C + 2], None, op.mult
            )
            nc.vector.tensor_add(t_l0[:], t_l0[:], t_l1[:])
            nc.sync.dma_start(o_lab[:], t_l0[:])

            # Select rows: mask = (idx >= s) - (idx >= e), then predicated
            # copy of the permuted-source tile over the own tile.
            if K > 0:
                t_idx = const.tile([PPART, FREE], dt.float32, tag="idx")
                nc.gpsimd.dma_start(t_idx[:], i_idx[:])
                t_bnd = const.tile([PPART, 2 * K], dt.float32, tag="bnd")
                nc.gpsimd.dma_start(t_bnd[:], i_bnd[:])
                t_own = panel.tile([PPART, K * FREE], dt.float32, tag="own")
                nc.sync.dma_start(t_own[:], i_sown[:])
                t_prm = panel.tile([PPART, K * FREE], dt.float32, tag="prm")
                nc.sync.dma_start(t_prm[:], i_sprm[:])
                for r in range(K):
                    cs = slice(r * FREE, (r + 1) * FREE)
                    m_e = work.tile([PPART, FREE], dt.float32, tag="m_e")
                    nc.vector.tensor_scalar(
                        m_e[:],
                        t_idx[:],
                        t_bnd[:, 2 * r + 1 : 2 * r + 2],
                        None,
                        op.is_ge,
                    )
                    m = work.tile([PPART, FREE], dt.int32, tag="m")
                    nc.vector.scalar_tensor_tensor(
                        m[:],
                        t_idx[:],
                        t_bnd[:, 2 * r : 2 * r + 1],
                        m_e[:],
                        op.is_ge,
                        op.subtract,
                    )
                    nc.vector.copy_predicated(t_own[:, cs], m[:], t_prm[:, cs])
                    nc.sync.dma_start(o_sel[:, cs], t_own[:, cs])
    nc.compile()
    return nc


def plan_and_maps(wave, onehot, lam, perm, dec, start):
    wave = np.ascontiguousarray(wave, dtype=np.float32)
    onehot = np.ascontiguousarray(onehot, dtype=np.float32)
    lam = np.asarray(lam, dtype=np.float32)
    perm = np.asarray(perm, dtype=np.int32)
    dec = np.asarray(dec, dtype=np.float32)
    start = np.asarray(start, dtype=np.int32)

    # --- host planning from the small control tensors ---
    crop = ((np.float32(1.0) - lam) * np.float32(L)).astype(np.int32)
    end = start + crop
    is_mix = (dec == 1.0) & (crop > 0)
    mix_rows = np.nonzero(is_mix)[0]
    idn_rows = np.nonzero(~is_mix)[0]
    T = len(mix_rows)
    K = -(-T // NCORES)  # ceil
    NI = ROWS_PER_CORE - K

    # Deal mix rows round-robin, then pad each core's select list with
    # identity rows (sentinel bounds keep them bit-exact identity copies).
    sel: list[list[int]] = [list(mix_rows[c::NCORES]) for c in range(NCORES)]
    pool = list(idn_rows)
    for c in range(NCORES):
        while len(sel[c]) < K:
            sel[c].append(pool.pop())
    idn = [[pool.pop() for _ in range(NI)] for _ in range(NCORES)]
    assert not pool

    idx_np = np.arange(L, dtype=np.float32).reshape(PPART, FREE)
    pmatT = np.zeros((C, C), dtype=np.float32)
    pmatT[perm, np.arange(C)] = 1.0
    c0 = np.float32(1.0) - dec + dec * lam
    c1 = dec * (np.float32(1.0) - lam)
    cv = np.stack([c0, c1], axis=1).astype(np.float32)

    in_maps = []
    for c in range(NCORES):
        m = {
            "idx": idx_np,
            "lbl": np.concatenate([onehot, pmatT, cv], axis=1),
        }
        if K > 0:
            rows = sel[c]
            m["sown"] = np.ascontiguousarray(
                wave[rows].reshape(K, PPART, FREE).transpose(1, 0, 2)
            ).reshape(PPART, K * FREE)
            m["sprm"] = np.ascontiguousarray(
                wave[perm[rows]].reshape(K, PPART, FREE).transpose(1, 0, 2)
            ).reshape(PPART, K * FREE)
            bnd = np.empty((PPART, 2 * K), dtype=np.float32)
            for j, b in enumerate(rows):
                if is_mix[b]:
                    bnd[:, 2 * j] = start[b]
                    bnd[:, 2 * j + 1] = end[b]
                else:
                    bnd[:, 2 * j] = BIG
                    bnd[:, 2 * j + 1] = BIG
            m["bnd"] = bnd
        if NI > 0:
            m["idn"] = wave[idn[c]].reshape(NI * PPART, FREE)
        in_maps.append(m)

    return K, NI, sel, idn, in_maps


def assemble(K, NI, sel, idn, per_core_outs):
    wave_mix = np.empty((B, L), dtype=np.float32)
    for c in range(NCORES):
        out = per_core_outs[c]
        if K > 0:
            o_sel = (
                np.asarray(out["o_sel"])
                .reshape(PPART, K, FREE)
                .transpose(1, 0, 2)
                .reshape(K, L)
            )
            for j, b in enumerate(sel[c]):
                wave_mix[b] = o_sel[j]
        if NI > 0:
            o_idn = np.asarray(out["o_idn"]).reshape(NI, L)
            for j, b in enumerate(idn[c]):
                wave_mix[b] = o_idn[j]
    label_out = np.asarray(per_core_outs[0]["o_lab"]).reshape(C, C)
    return wave_mix, label_out


def get_program(K):
    prog = _PROGRAM_CACHE.get(K)
    if prog is None:
        prog = _build_program(K)
        _PROGRAM_CACHE[K] = prog
    return prog


def kernel(wave, onehot, lam, perm, dec, start):
    global LAST_RESULTS
    K, NI, sel, idn, in_maps = plan_and_maps(wave, onehot, lam, perm, dec, start)
    prog = get_program(K)
    res = run_bass_kernel_spmd(prog, in_maps, core_ids=list(range(NCORES)))
    LAST_RESULTS = res
    return assemble(K, NI, sel, idn, res.results)
